# revision 1
# baseline (speedup 1.0000x reference)
"""Trainium2 Bass kernel for AdaptiveLRLinearWithChannel (moe_routing).

Reference math:
    w    = (weights_U[indices] @ weights_V).reshape(B, IN, OUT)
    out  = einsum('bni,bio->bno', x, w) + bias[indices]

Strategy (8 NeuronCores, data-parallel over B):
  - Shard B=256 into 8 x 32 batches; U/V/bias are tiny and are folded on host
    into per-batch weight matrices W[b] and bias rows (host marshalling only;
    all O(B*N*IN*OUT) FLOPs run on device).
  - Host re-lays x out as xT[b] = x[b].T so the contraction dim (IN) lands on
    SBUF partitions, and casts x/W to bf16 (matmul accumulates in f32 PSUM;
    rel err ~2e-3).
  - Per core: for each batch, 16 n-tiles of out[b] = xT[b].T @ W[b]: two
    K=128 matmuls accumulate into a [128, 256] PSUM tile; DVE adds the bias
    (broadcast across partitions) while evacuating PSUM -> SBUF as bf16;
    DMA out. Output upcast to f32 on host.
"""

import sys

for _p in ("/opt/trn_rl_repo",):
    if _p not in sys.path:
        sys.path.insert(0, _p)

import numpy as np

B = 256
N = 2048
IN_SZ = 256
OUT_SZ = 256
N_CORES = 8
BPC = B // N_CORES  # 32 batches per core
NT = N // 128  # 16 n-tiles per batch
KC = IN_SZ // 128  # 2 contraction chunks

_CACHE = {}


def _bf16():
    import ml_dtypes

    return ml_dtypes.bfloat16


def build_nc():
    """Build + compile the per-core Bass graph (same graph on all 8 cores)."""
    if "nc" in _CACHE:
        return _CACHE["nc"]

    import concourse.mybir as mybir
    import concourse.tile as tile
    from concourse import bacc

    nc = bacc.Bacc("TRN2", target_bir_lowering=False, debug=False)
    xT = nc.declare_dram_parameter(
        "xT", [BPC, IN_SZ, N], mybir.dt.bfloat16, isOutput=False
    )
    w = nc.declare_dram_parameter(
        "w", [BPC, IN_SZ, OUT_SZ], mybir.dt.bfloat16, isOutput=False
    )
    biasb = nc.declare_dram_parameter(
        "biasb", [BPC, 128, OUT_SZ], mybir.dt.float32, isOutput=False
    )
    out = nc.declare_dram_parameter(
        "out", [BPC, N, OUT_SZ], mybir.dt.bfloat16, isOutput=True
    )

    bf16 = mybir.dt.bfloat16
    f32 = mybir.dt.float32

    with tile.TileContext(nc) as tc:
        with (
            tc.tile_pool(name="xp", bufs=3) as xp,
            tc.tile_pool(name="wp", bufs=3) as wp,
            tc.tile_pool(name="bp", bufs=3) as bp,
            tc.tile_pool(name="op", bufs=8) as op,
            tc.tile_pool(name="psum", bufs=6, space="PSUM") as psum,
        ):
            for b in range(BPC):
                xt0 = xp.tile([128, N], bf16, tag="xt0")
                xt1 = xp.tile([128, N], bf16, tag="xt1")
                nc.sync.dma_start(out=xt0[:], in_=xT[b, 0:128, :])
                nc.sync.dma_start(out=xt1[:], in_=xT[b, 128:256, :])
                wt0 = wp.tile([128, OUT_SZ], bf16, tag="wt0")
                wt1 = wp.tile([128, OUT_SZ], bf16, tag="wt1")
                nc.sync.dma_start(out=wt0[:], in_=w[b, 0:128, :])
                nc.sync.dma_start(out=wt1[:], in_=w[b, 128:256, :])
                bt = bp.tile([128, OUT_SZ], f32, tag="bt")
                nc.sync.dma_start(out=bt[:], in_=biasb[b])

                for t in range(NT):
                    ps = psum.tile([128, OUT_SZ], f32, tag="ps")
                    sl = slice(t * 128, (t + 1) * 128)
                    nc.tensor.matmul(
                        ps[:], lhsT=xt0[:, sl], rhs=wt0[:], start=True, stop=False
                    )
                    nc.tensor.matmul(
                        ps[:], lhsT=xt1[:, sl], rhs=wt1[:], start=False, stop=True
                    )
                    ot = op.tile([128, OUT_SZ], bf16, tag="ot")
                    nc.vector.tensor_add(ot[:], ps[:], bt[:])
                    nc.sync.dma_start(out=out[b, sl, :], in_=ot[:])

    nc.compile()
    _CACHE["nc"] = nc
    return nc


def prep_in_maps(x, indices, weights_U, weights_V, bias):
    """Host-side marshalling: gather/synthesize per-batch weights, transpose
    x per batch, cast to bf16, shard along B."""
    bf16 = _bf16()
    x = np.asarray(x)
    idx = np.asarray(indices).astype(np.int64)
    U = np.asarray(weights_U, dtype=np.float32)
    V = np.asarray(weights_V, dtype=np.float32)
    bias = np.asarray(bias, dtype=np.float32)

    W = (U[idx] @ V).reshape(B, IN_SZ, OUT_SZ).astype(bf16)  # [B, in, out]
    xT = np.ascontiguousarray(x.transpose(0, 2, 1)).astype(bf16)  # [B, in, n]
    bias_sel = bias[idx][:, 0, :]  # [B, out] f32
    bias_bc = np.ascontiguousarray(
        np.broadcast_to(bias_sel[:, None, :], (B, 128, OUT_SZ)), dtype=np.float32
    )

    in_maps = []
    for c in range(N_CORES):
        s = slice(c * BPC, (c + 1) * BPC)
        in_maps.append({"xT": xT[s], "w": W[s], "biasb": bias_bc[s]})
    return in_maps


def assemble_output(results):
    out = np.concatenate(
        [np.asarray(results[c]["out"], dtype=np.float32) for c in range(N_CORES)],
        axis=0,
    )
    return out  # [B, N, OUT] f32


def kernel(x, indices, weights_U, weights_V, bias):
    from concourse import bass2jax

    nc = build_nc()
    in_maps = prep_in_maps(x, indices, weights_U, weights_V, bias)
    results = bass2jax.run_bass_via_pjrt(nc, in_maps, n_cores=N_CORES)
    return assemble_output(results)
